# revision 1
# baseline (speedup 1.0000x reference)
"""LDA-loss logits kernel for Trainium2 (8 NeuronCores, SPMD).

Computes logits[b, c] = -0.5 * ||feat[b] - centers[c]||^2
                      = feat[b]·centers[c] - 0.5||feat[b]||^2 - 0.5||centers[c]||^2

Strategy:
  - Shard centers (output columns) across the 8 cores: 10000 classes ->
    1250/core, padded to 1280 (= 2x512 + 256 matmul N-tiles).
  - Host prep: transpose feat/centers to [D, *] bf16 layouts (contraction on
    partitions), precompute the two squared-norm bias vectors in fp32.
  - Device: everything SBUF-resident. 8x128 K-chunks of bf16 matmuls
    accumulate in fp32 PSUM; eviction adds the per-row bias on ScalarE
    (activation Identity + per-partition bias) and the per-column bias on
    VectorE, then DMAs the fp32 tile out.
"""

import numpy as np
import ml_dtypes

BATCH = 4096
FEAT_DIM = 1024
NUM_CLASSES = 10000
N_CORES = 8
C_PER_REAL = NUM_CLASSES // N_CORES  # 1250
C_PER = 1280                         # padded per-core classes
P = 128
KO = FEAT_DIM // P                   # 8 contraction chunks
MT = BATCH // P                      # 32 output row tiles
N_TILES = ((0, 512), (512, 512), (1024, 256))

_NC = None


def _build_bass():
    import concourse.mybir as mybir
    import concourse.tile as tile
    from concourse import bacc

    nc = bacc.Bacc("TRN2", target_bir_lowering=False, debug=False)

    featT = nc.dram_tensor("featT", [FEAT_DIM, BATCH], mybir.dt.bfloat16,
                           kind="ExternalInput")
    centsT = nc.dram_tensor("centsT", [FEAT_DIM, C_PER], mybir.dt.bfloat16,
                            kind="ExternalInput")
    fsq = nc.dram_tensor("fsq", [P, MT], mybir.dt.float32, kind="ExternalInput")
    csqb = nc.dram_tensor("csqb", [P, C_PER], mybir.dt.float32,
                          kind="ExternalInput")
    out = nc.dram_tensor("out", [BATCH, C_PER], mybir.dt.float32,
                         kind="ExternalOutput")

    with tile.TileContext(nc) as tc:
        _lda_tile_kernel(tc, featT.ap(), centsT.ap(), fsq.ap(), csqb.ap(),
                         out.ap())
    nc.compile()
    return nc


def _lda_tile_kernel(tc, featT, centsT, fsq, csqb, out):
    import concourse.mybir as mybir

    nc = tc.nc
    featT_r = featT.rearrange("(ko p) b -> p ko b", p=P)
    centsT_r = centsT.rearrange("(ko p) c -> p ko c", p=P)
    out_r = out.rearrange("(mo p) c -> p mo c", p=P)

    with (
        tc.tile_pool(name="big", bufs=1) as big,
        tc.tile_pool(name="consts", bufs=1) as consts,
        tc.tile_pool(name="ostage", bufs=4) as ostage,
        tc.tile_pool(name="psum", bufs=4, space="PSUM") as psum,
    ):
        cent_sb = big.tile([P, KO, C_PER], mybir.dt.bfloat16)
        feat_sb = big.tile([P, KO, BATCH], mybir.dt.bfloat16)
        fsq_sb = consts.tile([P, MT], mybir.dt.float32)
        csq_sb = consts.tile([P, C_PER], mybir.dt.float32)

        nc.sync.dma_start(fsq_sb[:], fsq)
        nc.sync.dma_start(csq_sb[:], csqb)
        for k in range(KO):
            nc.sync.dma_start(cent_sb[:, k], centsT_r[:, k])
        # feat loads ordered m-range-major so early m-tiles are ready first
        MR = 4
        mr_size = BATCH // MR
        for mr in range(MR):
            sl = slice(mr * mr_size, (mr + 1) * mr_size)
            for k in range(KO):
                nc.sync.dma_start(feat_sb[:, k, sl], featT_r[:, k, sl])

        for m in range(MT):
            msl = slice(m * P, (m + 1) * P)
            for n0, nsz in N_TILES:
                ps = psum.tile([P, 512], mybir.dt.float32, tag="ps",
                               name="ps")[:, :nsz]
                for k in range(KO):
                    nc.tensor.matmul(
                        ps,
                        feat_sb[:, k, msl],
                        cent_sb[:, k, n0:n0 + nsz],
                        start=(k == 0),
                        stop=(k == KO - 1),
                    )
                ot = ostage.tile([P, 512], mybir.dt.float32, tag="ot",
                                 name="ot")[:, :nsz]
                # ot = psum + fsq[row]  (per-partition bias on ScalarE)
                nc.scalar.activation(
                    ot, ps, mybir.ActivationFunctionType.Identity,
                    bias=fsq_sb[:, m:m + 1],
                )
                # ot += csq[col]  (per-column bias on VectorE)
                nc.vector.tensor_add(ot, ot, csq_sb[:, n0:n0 + nsz])
                nc.sync.dma_start(out_r[:, m, n0:n0 + nsz], ot)


def _get_nc():
    global _NC
    if _NC is None:
        _NC = _build_bass()
    return _NC


def _prep_inputs(feat, centers):
    feat = np.asarray(feat, dtype=np.float32)
    centers = np.asarray(centers, dtype=np.float32)

    featT_bf = np.ascontiguousarray(feat.T).astype(ml_dtypes.bfloat16)
    fsq_v = -0.5 * np.einsum("bd,bd->b", feat, feat)
    fsq_mat = np.ascontiguousarray(fsq_v.reshape(MT, P).T)  # [P, MT]

    in_maps = []
    for i in range(N_CORES):
        cs = centers[i * C_PER_REAL:(i + 1) * C_PER_REAL]
        centsT_bf = np.zeros((FEAT_DIM, C_PER), dtype=ml_dtypes.bfloat16)
        centsT_bf[:, :C_PER_REAL] = cs.T.astype(ml_dtypes.bfloat16)
        csq = np.zeros(C_PER, dtype=np.float32)
        csq[:C_PER_REAL] = -0.5 * np.einsum("cd,cd->c", cs, cs)
        csqb = np.ascontiguousarray(
            np.broadcast_to(csq[None, :], (P, C_PER)))
        in_maps.append({
            "featT": featT_bf,
            "centsT": centsT_bf,
            "fsq": fsq_mat,
            "csqb": csqb,
        })
    return in_maps


def _run(inputs, trace=False, trace_cores=None):
    from concourse import bass_utils

    nc = _get_nc()
    in_maps = _prep_inputs(inputs["feat"], inputs["centers"])
    res = bass_utils.run_bass_kernel_spmd(
        nc, in_maps, core_ids=list(range(N_CORES)), trace=trace,
        trace_cores=trace_cores,
    )
    shards = [res.results[i]["out"][:, :C_PER_REAL] for i in range(N_CORES)]
    full = np.concatenate(shards, axis=1)
    return full, res


def kernel(**inputs) -> np.ndarray:
    return _run(inputs)[0]
